# revision 16
# baseline (speedup 1.0000x reference)
"""Sliding-window multi-head attention on 8 Trainium2 NeuronCores.

Sharding: tensor-parallel over heads. 16 heads -> 2 heads per core.
Each core computes q/k/v projections for its 2 heads (d' = 128 dims),
banded (window=256) attention for those heads over all tokens, and a
partial output projection (its 128 rows of Wo^T). Host sums the 8
partials and adds the bias.

v3 (token-major PV):
  - PV swaps operands: out u[128 q-tokens, 65] = ex2_slice.T @ [v|1]
    per 128-token q-block, using the full 128 output partitions (the
    old [65, 512] layout wasted half the PE array). Softmax normalize
    becomes a per-partition reciprocal[128,1] + tensor_scalar, killing
    the reciprocal/partition_broadcast/multiply chain.
  - attention output is token-major; cheap PE transposes ([128,128],
    128 cycles each) restore d-major for the output projection.
  - q windows are unpadded (256 wide at the edges); masking is two
    in-place triangular [128,128] multiplies on the exp output (the
    center block of each key-block window is always fully in-band).
  - chunk0/1 projections run e-major (4 q/k + 2 v psum banks open at
    once) so the PE saturates while x chunk pieces stream in; weights
    are DMAed in consumption order (wq, wv, x0, wk, x1, mpack, wo).
  - output drain works in (512-token, m-pair) units: two [128,512]
    matmuls + copies staged into one [128,1024] DMA that scatters to
    both m slabs via a 3D access pattern.
"""

import sys

sys.path.insert(0, "/opt/trn_rl_repo")

from contextlib import ExitStack

import numpy as np
from ml_dtypes import bfloat16

import concourse.bass as bass
import concourse.tile as tile
from concourse import bacc, mybir
from concourse.bass_utils import run_bass_kernel_spmd

F32 = mybir.dt.float32
F32R = mybir.dt.float32r
BF16 = mybir.dt.bfloat16
ACT_EXP = mybir.ActivationFunctionType.Exp
MUL = mybir.AluOpType.mult

N_CORES = 8
B, S, E = 2, 2048, 1024
H, D = 16, 64
T = B * S                # 4096 tokens total
NB = S // 128            # 16 key/query blocks per batch
WOFF = {"wq": 0, "wk": 1024, "wv": 2048, "wo": 3072}


class _Ctx:
    pass


def _emit(tc, io):
    nc = tc.nc
    with ExitStack() as ctx:
        const = ctx.enter_context(tc.tile_pool(name="const", bufs=1))
        big = ctx.enter_context(tc.tile_pool(name="big", bufs=1))
        xpool = ctx.enter_context(tc.tile_pool(name="xload", bufs=2))
        expool = ctx.enter_context(tc.tile_pool(name="expool", bufs=4))
        zpool = ctx.enter_context(tc.tile_pool(name="zpool", bufs=8))
        ostage = ctx.enter_context(tc.tile_pool(name="ostage", bufs=6))

        g = _Ctx()

        wpack = const.tile([128, 4096], BF16, tag="wpack")
        mpack = const.tile([128, 384], BF16, tag="mpack")

        g.w = lambda kind, e: wpack[:, WOFF[kind] + 128 * e : WOFF[kind] + 128 * e + 128]
        g.maskL = mpack[:, 0:128]
        g.maskR = mpack[:, 128:256]
        g.ident = mpack[:, 256:384]

        # ---- persistent activation buffers -----------------------------
        # vA blocks are 130 wide: [h0 d' (64) | ones | h1 d' (64) | ones]
        # so either head's PV rhs slice [.., 65] carries the ones column
        # (softmax denominator) at output column 64.
        g.qT = big.tile([128, T], BF16, tag="qT")
        g.kT = big.tile([128, T], BF16, tag="kT")
        g.vA = big.tile([128, 32 * 130], BF16, tag="vA")
        g.aoTM = big.tile([128, 32 * 128], BF16, tag="aoTM")  # token-major
        g.aoTd = big.tile([128, T], BF16, tag="aoTd")         # d-major
        vA3 = g.vA[:, 0 : 32 * 130].rearrange("p (blk c) -> p blk c", blk=32)
        nc.gpsimd.memset(vA3[:, :, 64:65], 1.0)
        nc.gpsimd.memset(vA3[:, :, 129:130], 1.0)

        # u accumulators (4 rotating [128,65] f32 slots) and transpose
        # staging (3 rotating [128,128] bf16 slots) share one psum bank;
        # allocated after the startup projection scope frees its banks.

        def _copy(sel, dst, src):
            # psum sources: only DVE/ACT may read PSUM (not gpsimd/Pool)
            if sel % 2:
                nc.scalar.copy(dst, src)
            else:
                nc.vector.tensor_copy(dst, src)

        # ---- output-projection drain: unit = (c, mpair) ----------------
        # c indexes 512-token groups over all of T; each unit computes two
        # m slabs' [128,512] partials and writes both in one DMA.
        g.wo_ready = []
        g.cnt = [0] * (T // 512)
        g.drain_i = 0

        def drain_wo(k, pools=None):
            pools = pools or [(psW, "w")]
            for _ in range(min(k, len(g.wo_ready))):
                c, mp = g.wo_ready.pop(0)
                ost = ostage.tile([128, 1024], BF16, tag="ost", name="ost")
                for i, m in enumerate((2 * mp, 2 * mp + 1)):
                    pool, ptag = pools[g.drain_i % len(pools)]
                    g.drain_i += 1
                    wps = pool.tile([128, 512], F32, tag=ptag, name="wps")
                    nc.tensor.matmul(
                        wps[:], g.w("wo", m), g.aoTd[:, 512 * c : 512 * c + 512],
                        start=True, stop=True,
                    )
                    _copy(c + i, ost[:, 512 * i : 512 * i + 512], wps[:])
                nc.sync.dma_start(
                    io["outT"][2 * mp : 2 * mp + 2, :, 512 * c : 512 * c + 512]
                    .rearrange("m p x -> p m x"),
                    ost[:].rearrange("p (m x) -> p m x", m=2),
                )

        # ---- projection loads ------------------------------------------
        def proj_loads(n):
            xt = xpool.tile([128, 8192], BF16, tag="xtc")
            for e in range(8):
                nc.sync.dma_start(
                    xt[:, 1024 * e : 1024 * e + 1024],
                    io["xT"][e, :, 1024 * n : 1024 * n + 1024],
                )
            return xt

        # ---- e-major projection for a chunk (startup; 6 psum banks).
        # q/k tiles are each a private bank so every e-piece feeds four
        # open accumulations (2048 PE cycles per 728ns x-piece arrival);
        # v runs i-major afterwards on two rotating private slots (PSUM
        # start=True zeroes a whole bank, so no bank sharing of live
        # accumulation groups).
        def proj_chunk_emajor(n, xt, qk, v0):
            qh = [qk.tile([128, 512], F32, tag=f"q{h}", name="qh") for h in range(2)]
            kh = [qk.tile([128, 512], F32, tag=f"k{h}", name="kh") for h in range(2)]
            for e in range(8):
                for half in range(2):
                    nc.tensor.matmul(
                        qh[half][:], g.w("wq", e),
                        xt[:, 1024 * e + 512 * half : 1024 * e + 512 * half + 512],
                        start=(e == 0), stop=(e == 7),
                    )
                for half in range(2):
                    nc.tensor.matmul(
                        kh[half][:], g.w("wk", e),
                        xt[:, 1024 * e + 512 * half : 1024 * e + 512 * half + 512],
                        start=(e == 0), stop=(e == 7),
                    )
            t0 = 1024 * n
            for half in range(2):
                nc.scalar.copy(g.qT[:, t0 + 512 * half : t0 + 512 * half + 512],
                               qh[half][:])
                nc.vector.tensor_copy(g.kT[:, t0 + 512 * half : t0 + 512 * half + 512],
                                      kh[half][:])
            for i in range(8):
                tb = 8 * n + i
                vps = v0.tile([128, 128], F32, tag="v", name="vps")
                for e in range(8):
                    nc.tensor.matmul(
                        vps[:], xt[:, 1024 * e + 128 * i : 1024 * e + 128 * i + 128],
                        g.w("wv", e),
                        start=(e == 0), stop=(e == 7),
                    )
                _copy(i, g.vA[:, 130 * tb : 130 * tb + 64], vps[:, 0:64])
                _copy(i, g.vA[:, 130 * tb + 65 : 130 * tb + 129], vps[:, 64:128])

        # ---- kind-major projection chunk (bg during attention) ---------
        def _chunk_steps_x(n, xt):
            for kind in ("wq", "wk"):
                for half in range(2):
                    t0 = 1024 * n + 512 * half
                    ps = psMix.tile([128, 512], F32, tag="p")
                    for e in range(8):
                        o = 1024 * e + 512 * half
                        nc.tensor.matmul(
                            ps[:], g.w(kind, e), xt[:, o : o + 512],
                            start=(e == 0), stop=(e == 7),
                        )
                    if kind == "wq":
                        nc.scalar.copy(g.qT[:, t0 : t0 + 512], ps[:])
                    else:
                        nc.vector.tensor_copy(g.kT[:, t0 : t0 + 512], ps[:])
                    yield
            for i in range(8):
                tb = 8 * n + i
                vps = psMix.tile([128, 128], F32, tag="p")
                for e in range(8):
                    o = 1024 * e + 128 * i
                    nc.tensor.matmul(
                        vps[:], xt[:, o : o + 128], g.w("wv", e),
                        start=(e == 0), stop=(e == 7),
                    )
                _copy(i, g.vA[:, 130 * tb : 130 * tb + 64], vps[:, 0:64])
                _copy(i, g.vA[:, 130 * tb + 65 : 130 * tb + 129], vps[:, 64:128])
                if i % 2:
                    yield

        # ---- attention stream for one (batch, head) ---------------------
        def finish(b, h, qb):
            u = g.uslot(qb)
            tb2 = NB * b + qb
            rz = zpool.tile([128, 1], F32R, tag="rz")
            with nc.allow_low_precision(reason="f32r is fp32-width"):
                nc.vector.reciprocal(rz[:], u[:, 64:65])
            dst = g.aoTM[:, 128 * tb2 + 64 * h : 128 * tb2 + 64 * h + 64]
            if qb % 2:
                nc.scalar.mul(dst, u[:, 0:64], rz[:].bitcast(F32))
            else:
                nc.vector.tensor_scalar(dst, u[:, 0:64], rz[:].bitcast(F32), None, MUL)
            if h == 1:
                tv = g.tslot(qb)
                nc.tensor.matmul(
                    tv, g.aoTM[:, 128 * tb2 : 128 * tb2 + 128], g.ident,
                    is_transpose=True, start=True, stop=True,
                    skip_group_check=True,
                )
                # (start=True is safe here: transposes replace, never
                # accumulate, and tbank holds only transpose slots)
                _copy(qb, g.aoTd[:, S * b + 128 * qb : S * b + 128 * qb + 128], tv)
                c = (S * b + 128 * qb) // 512
                g.cnt[c] += 1
                if g.cnt[c] == 4:
                    g.wo_ready.extend((c, mp) for mp in range(4))

        def attn(b, h, bg=None, bg_rate=1, drains=1):
            def bg_step(j):
                if bg is not None and (j % 2 == 0 or bg_rate > 1):
                    next(bg, None)
                drain_wo(drains, pools=[(psW, "w"), (psMix, "p")])

            def emit_sxm(j):
                q0 = 128 * max(j - 1, 0)
                W = min(128 * (j + 2), S) - q0
                with tc.high_priority(offset=60):
                    sT = psS.tile([128, 384], F32, tag="s")
                    nc.tensor.matmul(
                        sT[:, 0:W],
                        g.kT[64 * h : 64 * h + 64, S * b + 128 * j : S * b + 128 * j + 128],
                        g.qT[64 * h : 64 * h + 64, S * b + q0 : S * b + q0 + W],
                        start=True, stop=True,
                    )
                    ex2 = expool.tile([128, 384], BF16, tag="ex")
                    nc.scalar.activation(ex2[:, 0:W], sT[:, 0:W], ACT_EXP)
                    if j > 0:
                        meng = nc.gpsimd if j % 4 == 3 else nc.vector
                        meng.tensor_tensor(ex2[:, 0:128], ex2[:, 0:128], g.maskL, MUL)
                    if j < NB - 1:
                        meng = nc.gpsimd if j % 4 == 1 else nc.vector
                        meng.tensor_tensor(ex2[:, W - 128 : W], ex2[:, W - 128 : W],
                                           g.maskR, MUL)
                return ex2

            for qb in range(3):                      # slots for qb first seen at j<=1
                nc.vector.memset(g.uslot(qb), 0.0)
            ex2 = emit_sxm(0)
            for j in range(NB):
                ex2_next = emit_sxm(j + 1) if j + 1 < NB else None
                if j + 2 < NB:                       # slot for qb=j+2 (first PV at j+1)
                    nc.vector.memset(g.uslot(j + 2), 0.0)
                bg_step(j)
                tb = NB * b + j
                q0b = max(j - 1, 0)
                for qb in range(q0b, min(j + 2, NB)):
                    col0 = 128 * (qb - q0b)
                    nc.tensor.matmul(
                        g.uslot(qb),
                        ex2[:, col0 : col0 + 128],
                        g.vA[:, 130 * tb + 65 * h : 130 * tb + 65 * h + 65],
                        start=False,
                        stop=(j == min(qb + 1, NB - 1)),
                        skip_group_check=True,
                    )
                if j > 0:
                    finish(b, h, j - 1)
                if j == NB - 1:
                    finish(b, h, NB - 1)
                ex2 = ex2_next
            if bg is not None:
                for _ in bg:
                    pass

        # ---- schedule ---------------------------------------------------
        # DMA order: wq, wv, x0, wk, x1, mpack, wo, x2, x3 (consumption
        # order; x pieces land while the PE chews the previous ones).
        nc.sync.dma_start(wpack[:, 0:1024], io["wpack"][:, 0:1024])
        nc.sync.dma_start(wpack[:, 1024:2048], io["wpack"][:, 1024:2048])
        xt0 = proj_loads(0)
        nc.sync.dma_start(wpack[:, 2048:3072], io["wpack"][:, 2048:3072])
        xt1 = proj_loads(1)
        nc.sync.dma_start(mpack[:], io["mpack"][:])
        nc.sync.dma_start(wpack[:, 3072:4096], io["wpack"][:, 3072:4096])

        with tc.tile_pool(name="qk0", bufs=1, space="PSUM") as qk, \
             tc.tile_pool(name="v0", bufs=2, space="PSUM") as v0:
            proj_chunk_emajor(0, xt0, qk, v0)
            proj_chunk_emajor(1, xt1, qk, v0)

        psMix = ctx.enter_context(tc.tile_pool(name="psMix", bufs=2, space="PSUM"))
        psS = ctx.enter_context(tc.tile_pool(name="psS", bufs=2, space="PSUM"))
        psW = ctx.enter_context(tc.tile_pool(name="psW", bufs=2, space="PSUM"))
        upool = ctx.enter_context(tc.tile_pool(name="upool", bufs=1, space="PSUM"))
        tpool = ctx.enter_context(tc.tile_pool(name="tpool", bufs=1, space="PSUM"))
        # u accumulators: 4 rotating [128,65] slots in ONE psum bank.
        # start=True zeroes a whole 2KB bank, so shared-bank groups must
        # never use it: slots are memset then accumulated with
        # start=False only. Transposes (replace-only writes) rotate two
        # slots of their own bank and may use start=True.
        ubank = upool.tile([128, 512], F32, tag="ubank")
        tbank = tpool.tile([128, 256], BF16, tag="tbank")
        g.uslot = lambda qb: ubank[:, 65 * (qb % 4) : 65 * (qb % 4) + 65]
        g.tslot = lambda i: tbank[:, 128 * (i % 2) : 128 * (i % 2) + 128]

        xt2 = proj_loads(2)
        xt3 = proj_loads(3)
        attn(0, 0, bg=_chunk_steps_x(2, xt2), bg_rate=2)
        attn(0, 1, bg=_chunk_steps_x(3, xt3), bg_rate=2, drains=1)
        attn(1, 0, drains=2)
        attn(1, 1, drains=2)
        drain_wo(64, pools=[(psW, "w"), (psMix, "p")])
        if "dbg" in io:
            nc.sync.dma_start(io["dbg"][:, 0:T], g.qT[:])
            nc.sync.dma_start(io["dbg"][:, T : 2 * T], g.kT[:])
            nc.sync.dma_start(io["dbg"][:, 2 * T : 2 * T + 32 * 128], g.aoTM[:])
            nc.sync.dma_start(io["dbg"][:, 3 * T : 4 * T], g.aoTd[:])
            nc.sync.dma_start(io["dbg"][:, 4 * T : 4 * T + 32 * 130], g.vA[:])


def build_program(dbg=False):
    nc = bacc.Bacc("TRN2", target_bir_lowering=False, debug=False, num_devices=N_CORES)
    io = {}

    def inp(name, shape):
        io[name] = nc.dram_tensor(name, shape, BF16, kind="ExternalInput").ap()

    inp("xT", [8, 128, T])
    inp("wpack", [128, 4096])
    inp("mpack", [128, 384])
    io["outT"] = nc.dram_tensor("outT", [8, 128, T], BF16, kind="ExternalOutput").ap()
    if dbg:
        io["dbg"] = nc.dram_tensor("dbg", [128, 4 * T + 32 * 130], BF16, kind="ExternalOutput").ap()

    with tile.TileContext(nc) as tc:
        _emit(tc, io)
    nc.compile()
    return nc


def _host_inputs(x, Wq, Wk, Wv, Wo):
    """Per-core input maps (host-side sharding / relayout)."""
    xf = np.ascontiguousarray(x.reshape(T, E).T).astype(bfloat16)  # [1024, 4096]
    xT = xf.reshape(8, 128, T)

    # triangular edge masks: for key r, query col c within a 128 block
    #   left block  (q block j-1): valid iff r <= c
    #   right block (q block j+1): valid iff r >= c
    r = np.arange(128)[:, None]
    c = np.arange(128)[None, :]
    m_left = (r <= c).astype(np.float32)
    m_right = (r >= c).astype(np.float32)
    mpack = np.concatenate(
        [m_left, m_right, np.eye(128, dtype=np.float32)], axis=1
    ).astype(bfloat16)

    scale = 1.0 / np.sqrt(D)
    in_maps = []
    for core in range(N_CORES):
        rows = slice(128 * core, 128 * core + 128)
        wq = np.ascontiguousarray((Wq[rows, :] * scale).T)   # [1024 e, 128 d']
        wk = np.ascontiguousarray(Wk[rows, :].T)
        wv = np.ascontiguousarray(Wv[rows, :].T)
        wqc = wq.reshape(8, 128, 128)
        wkc = wk.reshape(8, 128, 128)
        wvc = wv.reshape(8, 128, 128)
        woc = Wo[:, rows].T.reshape(128, 8, 128).transpose(1, 0, 2)  # [8,128 d',128 e]
        wpack = np.zeros((128, 4096), dtype=np.float32)
        for e in range(8):
            wpack[:, 0 + 128 * e : 128 * e + 128] = wqc[e]
            wpack[:, 1024 + 128 * e : 1152 + 128 * e] = wkc[e]
            wpack[:, 2048 + 128 * e : 2176 + 128 * e] = wvc[e]
            wpack[:, 3072 + 128 * e : 3200 + 128 * e] = woc[e]
        in_maps.append(
            {"xT": xT, "wpack": wpack.astype(bfloat16), "mpack": mpack}
        )
    return in_maps


_NC_CACHE = None


def kernel(x, Wq, Wk, Wv, Wo, bo):
    global _NC_CACHE
    x = np.asarray(x, dtype=np.float32)
    Wq = np.asarray(Wq, dtype=np.float32)
    Wk = np.asarray(Wk, dtype=np.float32)
    Wv = np.asarray(Wv, dtype=np.float32)
    Wo = np.asarray(Wo, dtype=np.float32)
    bo = np.asarray(bo, dtype=np.float32)

    if _NC_CACHE is None:
        _NC_CACHE = build_program()
    nc = _NC_CACHE

    in_maps = _host_inputs(x, Wq, Wk, Wv, Wo)
    res = run_bass_kernel_spmd(nc, in_maps, core_ids=list(range(N_CORES)))

    acc = np.zeros((E, T), dtype=np.float32)
    for c in range(N_CORES):
        acc += res.results[c]["outT"].astype(np.float32).reshape(E, T)
    out = acc.T + bo[None, :]
    return np.ascontiguousarray(out.reshape(B, S, E))


# revision 17
# speedup vs baseline: 1.1851x; 1.1851x over previous
"""Sliding-window multi-head attention on 8 Trainium2 NeuronCores.

Sharding: tensor-parallel over heads. 16 heads -> 2 heads per core.
Each core computes q/k/v projections for its 2 heads (d' = 128 dims),
banded (window=256) attention for those heads over all tokens, and a
partial output projection (its 128 rows of Wo^T). Host sums the 8
partials and adds the bias.

v3 (token-major PV):
  - PV swaps operands: out u[128 q-tokens, 65] = ex2_slice.T @ [v|1]
    per 128-token q-block, using the full 128 output partitions (the
    old [65, 512] layout wasted half the PE array). Softmax normalize
    becomes a per-partition reciprocal[128,1] + tensor_scalar, killing
    the reciprocal/partition_broadcast/multiply chain.
  - attention output is token-major; cheap PE transposes ([128,128],
    128 cycles each) restore d-major for the output projection.
  - q windows are unpadded (256 wide at the edges); masking is two
    in-place triangular [128,128] multiplies on the exp output (the
    center block of each key-block window is always fully in-band).
  - chunk0/1 projections run e-major (4 q/k + 2 v psum banks open at
    once) so the PE saturates while x chunk pieces stream in; weights
    are DMAed in consumption order (wq, wv, x0, wk, x1, mpack, wo).
  - output drain works in (512-token, m-pair) units: two [128,512]
    matmuls + copies staged into one [128,1024] DMA that scatters to
    both m slabs via a 3D access pattern.
"""

import sys

sys.path.insert(0, "/opt/trn_rl_repo")

from contextlib import ExitStack

import numpy as np
from ml_dtypes import bfloat16

import concourse.bass as bass
import concourse.tile as tile
from concourse import bacc, mybir
from concourse.bass_utils import run_bass_kernel_spmd

F32 = mybir.dt.float32
F32R = mybir.dt.float32r
BF16 = mybir.dt.bfloat16
ACT_EXP = mybir.ActivationFunctionType.Exp
MUL = mybir.AluOpType.mult

N_CORES = 8
B, S, E = 2, 2048, 1024
H, D = 16, 64
T = B * S                # 4096 tokens total
NB = S // 128            # 16 key/query blocks per batch
WOFF = {"wq": 0, "wk": 1024, "wv": 2048, "wo": 3072}


class _Ctx:
    pass


def _emit(tc, io):
    nc = tc.nc
    with ExitStack() as ctx:
        const = ctx.enter_context(tc.tile_pool(name="const", bufs=1))
        big = ctx.enter_context(tc.tile_pool(name="big", bufs=1))
        xpool = ctx.enter_context(tc.tile_pool(name="xload", bufs=2))
        expool = ctx.enter_context(tc.tile_pool(name="expool", bufs=4))
        zpool = ctx.enter_context(tc.tile_pool(name="zpool", bufs=8))
        ostage = ctx.enter_context(tc.tile_pool(name="ostage", bufs=6))

        g = _Ctx()

        wpack = const.tile([128, 4096], BF16, tag="wpack")
        mpack = const.tile([128, 384], BF16, tag="mpack")

        g.w = lambda kind, e: wpack[:, WOFF[kind] + 128 * e : WOFF[kind] + 128 * e + 128]
        g.maskL = mpack[:, 0:128]
        g.maskR = mpack[:, 128:256]
        g.ident = mpack[:, 256:384]

        # ---- persistent activation buffers -----------------------------
        # vA blocks are 130 wide: [h0 d' (64) | ones | h1 d' (64) | ones]
        # so either head's PV rhs slice [.., 65] carries the ones column
        # (softmax denominator) at output column 64.
        g.qT = big.tile([128, T], BF16, tag="qT")
        g.kT = big.tile([128, T], BF16, tag="kT")
        g.vA = big.tile([128, 32 * 130], BF16, tag="vA")
        g.aoTM = big.tile([128, 32 * 128], BF16, tag="aoTM")  # token-major
        g.aoTd = big.tile([128, T], BF16, tag="aoTd")         # d-major
        vA3 = g.vA[:, 0 : 32 * 130].rearrange("p (blk c) -> p blk c", blk=32)
        nc.gpsimd.memset(vA3[:, :, 64:65], 1.0)
        nc.gpsimd.memset(vA3[:, :, 129:130], 1.0)

        # u accumulators (4 rotating [128,65] f32 slots) and transpose
        # staging (3 rotating [128,128] bf16 slots) share one psum bank;
        # allocated after the startup projection scope frees its banks.

        def _copy(sel, dst, src):
            # psum sources: only DVE/ACT may read PSUM (not gpsimd/Pool)
            if sel % 2:
                nc.scalar.copy(dst, src)
            else:
                nc.vector.tensor_copy(dst, src)

        # ---- output-projection drain: unit = (c, mpair) ----------------
        # c indexes 512-token groups over all of T; each unit computes two
        # m slabs' [128,512] partials and writes both in one DMA.
        g.wo_ready = []
        g.cnt = [0] * (T // 512)
        g.drain_i = 0

        def drain_wo(k, pools=None):
            pools = pools or [(psW, "w")]
            for _ in range(min(k, len(g.wo_ready))):
                c, mp = g.wo_ready.pop(0)
                ost = ostage.tile([128, 1024], BF16, tag="ost", name="ost")
                for i, m in enumerate((2 * mp, 2 * mp + 1)):
                    pool, ptag = pools[g.drain_i % len(pools)]
                    g.drain_i += 1
                    wps = pool.tile([128, 512], F32, tag=ptag, name="wps")
                    nc.tensor.matmul(
                        wps[:], g.w("wo", m), g.aoTd[:, 512 * c : 512 * c + 512],
                        start=True, stop=True,
                    )
                    _copy(c + i, ost[:, 512 * i : 512 * i + 512], wps[:])
                nc.sync.dma_start(
                    io["outT"][2 * mp : 2 * mp + 2, :, 512 * c : 512 * c + 512]
                    .rearrange("m p x -> p m x"),
                    ost[:].rearrange("p (m x) -> p m x", m=2),
                )

        # ---- projection loads ------------------------------------------
        def proj_loads(n):
            xt = xpool.tile([128, 8192], BF16, tag="xtc")
            for e in range(8):
                nc.sync.dma_start(
                    xt[:, 1024 * e : 1024 * e + 1024],
                    io["xT"][e, :, 1024 * n : 1024 * n + 1024],
                )
            return xt

        # ---- e-major projection for a chunk (startup; 6 psum banks).
        # q/k tiles are each a private bank so every e-piece feeds four
        # open accumulations (2048 PE cycles per 728ns x-piece arrival);
        # v runs i-major afterwards on two rotating private slots (PSUM
        # start=True zeroes a whole bank, so no bank sharing of live
        # accumulation groups).
        def proj_chunk_emajor(n, xt, qk, v0):
            qh = [qk.tile([128, 512], F32, tag=f"q{h}", name="qh") for h in range(2)]
            kh = [qk.tile([128, 512], F32, tag=f"k{h}", name="kh") for h in range(2)]
            for e in range(8):
                for half in range(2):
                    nc.tensor.matmul(
                        qh[half][:], g.w("wq", e),
                        xt[:, 1024 * e + 512 * half : 1024 * e + 512 * half + 512],
                        start=(e == 0), stop=(e == 7),
                    )
                for half in range(2):
                    nc.tensor.matmul(
                        kh[half][:], g.w("wk", e),
                        xt[:, 1024 * e + 512 * half : 1024 * e + 512 * half + 512],
                        start=(e == 0), stop=(e == 7),
                    )
            t0 = 1024 * n
            for half in range(2):
                nc.scalar.copy(g.qT[:, t0 + 512 * half : t0 + 512 * half + 512],
                               qh[half][:])
                nc.vector.tensor_copy(g.kT[:, t0 + 512 * half : t0 + 512 * half + 512],
                                      kh[half][:])
            for i in range(8):
                tb = 8 * n + i
                vps = v0.tile([128, 128], F32, tag="v", name="vps")
                for e in range(8):
                    nc.tensor.matmul(
                        vps[:], xt[:, 1024 * e + 128 * i : 1024 * e + 128 * i + 128],
                        g.w("wv", e),
                        start=(e == 0), stop=(e == 7),
                    )
                _copy(i, g.vA[:, 130 * tb : 130 * tb + 64], vps[:, 0:64])
                _copy(i, g.vA[:, 130 * tb + 65 : 130 * tb + 129], vps[:, 64:128])

        # ---- kind-major projection chunk (bg during attention) ---------
        def _chunk_steps_x(n, xt):
            for kind in ("wq", "wk"):
                for half in range(2):
                    t0 = 1024 * n + 512 * half
                    ps = psMix.tile([128, 512], F32, tag="p")
                    for e in range(8):
                        o = 1024 * e + 512 * half
                        nc.tensor.matmul(
                            ps[:], g.w(kind, e), xt[:, o : o + 512],
                            start=(e == 0), stop=(e == 7),
                        )
                    if kind == "wq":
                        nc.scalar.copy(g.qT[:, t0 : t0 + 512], ps[:])
                    else:
                        nc.vector.tensor_copy(g.kT[:, t0 : t0 + 512], ps[:])
                    yield
            for i in range(8):
                tb = 8 * n + i
                vps = psMix.tile([128, 128], F32, tag="p")
                for e in range(8):
                    o = 1024 * e + 128 * i
                    nc.tensor.matmul(
                        vps[:], xt[:, o : o + 128], g.w("wv", e),
                        start=(e == 0), stop=(e == 7),
                    )
                _copy(i, g.vA[:, 130 * tb : 130 * tb + 64], vps[:, 0:64])
                _copy(i, g.vA[:, 130 * tb + 65 : 130 * tb + 129], vps[:, 64:128])
                if i % 2:
                    yield

        # ---- attention stream: one pass per batch, heads interleaved ----
        def finish(b, h, qb):
            u = g.uslot(qb, h)
            tb2 = NB * b + qb
            rz = zpool.tile([128, 1], F32R, tag="rz")
            with nc.allow_low_precision(reason="f32r is fp32-width"):
                nc.vector.reciprocal(rz[:], u[:, 64:65])
            dst = g.aoTM[:, 128 * tb2 + 64 * h : 128 * tb2 + 64 * h + 64]
            nc.vector.tensor_scalar(dst, u[:, 0:64], rz[:].bitcast(F32), None, MUL)
            if h == 1:
                tv = g.tslot(qb)
                nc.tensor.matmul(
                    tv, g.aoTM[:, 128 * tb2 : 128 * tb2 + 128], g.ident,
                    is_transpose=True, start=True, stop=True,
                    skip_group_check=True,
                )
                _copy(qb, g.aoTd[:, S * b + 128 * qb : S * b + 128 * qb + 128], tv)
                c = (S * b + 128 * qb) // 512
                g.cnt[c] += 1
                if g.cnt[c] == 4:
                    g.wo_ready.extend((c, mp) for mp in range(4))

        def attn_pass(b, bg=None, bg_rate=1):
            def bg_step(j):
                if bg is not None and (j % 2 == 0 or bg_rate > 1):
                    next(bg, None)
                k = 2 if len(g.wo_ready) >= 6 else 1
                drain_wo(k, pools=[(psW, "w"), (psMix, "p")])

            def emit_sxm(j, h):
                q0 = 128 * max(j - 1, 0)
                W = min(128 * (j + 2), S) - q0
                with tc.high_priority(offset=60):
                    sT = psS.tile([128, 384], F32, tag="s")
                    nc.tensor.matmul(
                        sT[:, 0:W],
                        g.kT[64 * h : 64 * h + 64, S * b + 128 * j : S * b + 128 * j + 128],
                        g.qT[64 * h : 64 * h + 64, S * b + q0 : S * b + q0 + W],
                        start=True, stop=True,
                    )
                    ex2 = expool.tile([128, 384], BF16, tag="ex")
                    nc.scalar.activation(ex2[:, 0:W], sT[:, 0:W], ACT_EXP)
                    if j > 0:
                        meng = nc.gpsimd if (2 * j + h) % 4 == 3 else nc.vector
                        meng.tensor_tensor(ex2[:, 0:128], ex2[:, 0:128], g.maskL, MUL)
                    if j < NB - 1:
                        meng = nc.gpsimd if (2 * j + h) % 4 == 1 else nc.vector
                        meng.tensor_tensor(ex2[:, W - 128 : W], ex2[:, W - 128 : W],
                                           g.maskR, MUL)
                return ex2

            nc.vector.memset(ubank[:, 0:260], 0.0)      # qb 0,1 slots (both heads)
            ex2 = [emit_sxm(0, 0), emit_sxm(0, 1)]
            for j in range(NB):
                ex2_next = [emit_sxm(j + 1, 0), emit_sxm(j + 1, 1)] if j + 1 < NB else None
                bg_step(j)
                tb = NB * b + j
                q0b = max(j - 1, 0)
                for h in range(2):
                    for qb in range(q0b, min(j + 2, NB)):
                        col0 = 128 * (qb - q0b)
                        nc.tensor.matmul(
                            g.uslot(qb, h),
                            ex2[h][:, col0 : col0 + 128],
                            g.vA[:, 130 * tb + 65 * h : 130 * tb + 65 * h + 65],
                            start=False,
                            stop=(j == min(qb + 1, NB - 1)),
                            skip_group_check=True,
                        )
                for qb in ([j - 1] if j > 0 else []) + ([NB - 1] if j == NB - 1 else []):
                    finish(b, 0, qb)
                    finish(b, 1, qb)
                if j + 2 < NB:
                    # zero qb=j+2's slot pair (reuses qb=j-1's, read above)
                    nc.vector.memset(ubank[:, 130 * ((j + 2) % 3) : 130 * ((j + 2) % 3) + 130], 0.0)
                ex2 = ex2_next
            if bg is not None:
                for _ in bg:
                    pass

        # ---- schedule ---------------------------------------------------
        # DMA order: wq, wv, x0, wk, x1, mpack, wo, x2, x3 (consumption
        # order; x pieces land while the PE chews the previous ones).
        nc.sync.dma_start(wpack[:, 0:1024], io["wpack"][:, 0:1024])
        nc.sync.dma_start(wpack[:, 1024:2048], io["wpack"][:, 1024:2048])
        xt0 = proj_loads(0)
        nc.sync.dma_start(wpack[:, 2048:3072], io["wpack"][:, 2048:3072])
        xt1 = proj_loads(1)
        nc.sync.dma_start(mpack[:], io["mpack"][:])
        nc.sync.dma_start(wpack[:, 3072:4096], io["wpack"][:, 3072:4096])

        with tc.tile_pool(name="qk0", bufs=1, space="PSUM") as qk, \
             tc.tile_pool(name="v0", bufs=2, space="PSUM") as v0:
            proj_chunk_emajor(0, xt0, qk, v0)
            proj_chunk_emajor(1, xt1, qk, v0)

        psMix = ctx.enter_context(tc.tile_pool(name="psMix", bufs=2, space="PSUM"))
        psS = ctx.enter_context(tc.tile_pool(name="psS", bufs=2, space="PSUM"))
        psW = ctx.enter_context(tc.tile_pool(name="psW", bufs=2, space="PSUM"))
        upool = ctx.enter_context(tc.tile_pool(name="upool", bufs=1, space="PSUM"))
        tpool = ctx.enter_context(tc.tile_pool(name="tpool", bufs=1, space="PSUM"))
        # u accumulators: 4 rotating [128,65] slots in ONE psum bank.
        # start=True zeroes a whole 2KB bank, so shared-bank groups must
        # never use it: slots are memset then accumulated with
        # start=False only. Transposes (replace-only writes) rotate two
        # slots of their own bank and may use start=True.
        ubank = upool.tile([128, 512], F32, tag="ubank")
        tbank = tpool.tile([128, 256], BF16, tag="tbank")
        g.uslot = lambda qb, h: ubank[:, 130 * (qb % 3) + 65 * h : 130 * (qb % 3) + 65 * h + 65]
        g.tslot = lambda i: tbank[:, 128 * (i % 2) : 128 * (i % 2) + 128]

        xt2 = proj_loads(2)
        xt3 = proj_loads(3)
        from itertools import chain as _chain
        attn_pass(0, bg=_chain(_chunk_steps_x(2, xt2), _chunk_steps_x(3, xt3)),
                  bg_rate=2)
        attn_pass(1)
        drain_wo(64, pools=[(psW, "w"), (psMix, "p")])
        if "dbg" in io:
            nc.sync.dma_start(io["dbg"][:, 0:T], g.qT[:])
            nc.sync.dma_start(io["dbg"][:, T : 2 * T], g.kT[:])
            nc.sync.dma_start(io["dbg"][:, 2 * T : 2 * T + 32 * 128], g.aoTM[:])
            nc.sync.dma_start(io["dbg"][:, 3 * T : 4 * T], g.aoTd[:])
            nc.sync.dma_start(io["dbg"][:, 4 * T : 4 * T + 32 * 130], g.vA[:])


def build_program(dbg=False):
    nc = bacc.Bacc("TRN2", target_bir_lowering=False, debug=False, num_devices=N_CORES)
    io = {}

    def inp(name, shape):
        io[name] = nc.dram_tensor(name, shape, BF16, kind="ExternalInput").ap()

    inp("xT", [8, 128, T])
    inp("wpack", [128, 4096])
    inp("mpack", [128, 384])
    io["outT"] = nc.dram_tensor("outT", [8, 128, T], BF16, kind="ExternalOutput").ap()
    if dbg:
        io["dbg"] = nc.dram_tensor("dbg", [128, 4 * T + 32 * 130], BF16, kind="ExternalOutput").ap()

    with tile.TileContext(nc) as tc:
        _emit(tc, io)
    nc.compile()
    return nc


def _host_inputs(x, Wq, Wk, Wv, Wo):
    """Per-core input maps (host-side sharding / relayout)."""
    xf = np.ascontiguousarray(x.reshape(T, E).T).astype(bfloat16)  # [1024, 4096]
    xT = xf.reshape(8, 128, T)

    # triangular edge masks: for key r, query col c within a 128 block
    #   left block  (q block j-1): valid iff r <= c
    #   right block (q block j+1): valid iff r >= c
    r = np.arange(128)[:, None]
    c = np.arange(128)[None, :]
    m_left = (r <= c).astype(np.float32)
    m_right = (r >= c).astype(np.float32)
    mpack = np.concatenate(
        [m_left, m_right, np.eye(128, dtype=np.float32)], axis=1
    ).astype(bfloat16)

    scale = 1.0 / np.sqrt(D)
    in_maps = []
    for core in range(N_CORES):
        rows = slice(128 * core, 128 * core + 128)
        wq = np.ascontiguousarray((Wq[rows, :] * scale).T)   # [1024 e, 128 d']
        wk = np.ascontiguousarray(Wk[rows, :].T)
        wv = np.ascontiguousarray(Wv[rows, :].T)
        wqc = wq.reshape(8, 128, 128)
        wkc = wk.reshape(8, 128, 128)
        wvc = wv.reshape(8, 128, 128)
        woc = Wo[:, rows].T.reshape(128, 8, 128).transpose(1, 0, 2)  # [8,128 d',128 e]
        wpack = np.zeros((128, 4096), dtype=np.float32)
        for e in range(8):
            wpack[:, 0 + 128 * e : 128 * e + 128] = wqc[e]
            wpack[:, 1024 + 128 * e : 1152 + 128 * e] = wkc[e]
            wpack[:, 2048 + 128 * e : 2176 + 128 * e] = wvc[e]
            wpack[:, 3072 + 128 * e : 3200 + 128 * e] = woc[e]
        in_maps.append(
            {"xT": xT, "wpack": wpack.astype(bfloat16), "mpack": mpack}
        )
    return in_maps


_NC_CACHE = None


def kernel(x, Wq, Wk, Wv, Wo, bo):
    global _NC_CACHE
    x = np.asarray(x, dtype=np.float32)
    Wq = np.asarray(Wq, dtype=np.float32)
    Wk = np.asarray(Wk, dtype=np.float32)
    Wv = np.asarray(Wv, dtype=np.float32)
    Wo = np.asarray(Wo, dtype=np.float32)
    bo = np.asarray(bo, dtype=np.float32)

    if _NC_CACHE is None:
        _NC_CACHE = build_program()
    nc = _NC_CACHE

    in_maps = _host_inputs(x, Wq, Wk, Wv, Wo)
    res = run_bass_kernel_spmd(nc, in_maps, core_ids=list(range(N_CORES)))

    acc = np.zeros((E, T), dtype=np.float32)
    for c in range(N_CORES):
        acc += res.results[c]["outT"].astype(np.float32).reshape(E, T)
    out = acc.T + bo[None, :]
    return np.ascontiguousarray(out.reshape(B, S, E))
